# revision 30
# baseline (speedup 1.0000x reference)
"""Single-head causal attention on 8 TRN2 NeuronCores, data-parallel over batch.

Problem: x [512, 256, 384] f32, Wq/Wk/Wv [384, 64] f32.
  q/k/v = x @ W;  S = q k^T / sqrt(384); causal softmax; out = P v.

Sharding: batch 512 -> 64 per core.  Host pre-transposes x so each device DMA
is fully contiguous; weights are replicated (tiny).

Device algorithm (per pair of batches), v2 -- tuned to keep the PE array
saturated (no micro-idles -> HAM stays at K=8/8):
  - qkT [128, 2, 256] = [Wq*scale | Wk]^T-stationary matmul over xT;
    rows 0:64 = q^T (h on partitions), rows 64:128 = k^T.
  - v [s, h] computed DIRECTLY with x^T chunks as the stationary operand and
    Wv streaming (12 tiny matmuls) -- no PE transposes, no vT cast.
    A ones-column is appended (memset once per buffer slot) so the PV matmul
    emits the softmax row-sum in column 64 for free.
  - k bounced to base partition 0 via I_64 matmul, bf16 PSUM output.
  - ST per batch in 3 [128,128] blocks [diag0 | diag1 | off]: one batched
    exp (ACT), one affine_select (gpsimd) over the two adjacent diag blocks.
  - PV accumulates [t, 65] per t-block; normalization happens ON HOST
    (out[..., :64] / out[..., 64:65]), so the device just casts PSUM->bf16
    and DMAs -- no reciprocal / per-partition scaling ops.
"""

import numpy as np

import concourse.bacc as bacc
import concourse.bass as bass
import concourse.mybir as mybir
import concourse.tile as tile
from concourse.bass_utils import run_bass_kernel_spmd

N_CORES = 8
B, T, C, H = 512, 256, 384, 64
BPC = B // N_CORES          # 64 batches per core
PAIRS = BPC // 2            # 32 pair-iterations per core
NCHUNK = C // 128           # 3 contraction chunks
SCALE = 1.0 / np.sqrt(C)    # note: reference scales by C**-0.5, not H**-0.5

F32 = mybir.dt.float32
BF16 = mybir.dt.bfloat16
EXP = mybir.ActivationFunctionType.Exp


def build_bass():
    nc = bacc.Bacc(None, target_bir_lowering=False, debug=False)
    x_in = nc.dram_tensor("xt", [PAIRS, 128, NCHUNK, 2, T], BF16, kind="ExternalInput")
    wqk_in = nc.dram_tensor("wqk", [128, NCHUNK, 128], BF16, kind="ExternalInput")
    wv_in = nc.dram_tensor("wv", [128, NCHUNK, H], BF16, kind="ExternalInput")
    out_d = nc.dram_tensor("out", [PAIRS, 128, 2, 2, H + 1], BF16, kind="ExternalOutput")

    with tile.TileContext(nc) as tc:
        with (
            tc.tile_pool(name="const", bufs=1) as const_pool,
            tc.tile_pool(name="xt", bufs=5) as xt_pool,
            tc.tile_pool(name="proj_sb", bufs=4) as proj_sb,
            tc.tile_pool(name="v_sb", bufs=4) as v_sb_pool,
            tc.tile_pool(name="p_sb", bufs=4) as p_pool,
            tc.tile_pool(name="o_sb", bufs=3) as o_sb_pool,
            tc.tile_pool(name="proj_ps", bufs=2, space="PSUM") as proj_ps,
            tc.tile_pool(name="kb_ps", bufs=2, space="PSUM") as kb_ps,
            tc.tile_pool(name="v_ps", bufs=1, space="PSUM") as v_ps_pool,
            tc.tile_pool(name="st_ps", bufs=2, space="PSUM") as st_ps_pool,
            tc.tile_pool(name="o_ps", bufs=1, space="PSUM") as o_ps_pool,
        ):
            wqk = const_pool.tile([128, NCHUNK, 128], BF16)
            nc.sync.dma_start(wqk[:], wqk_in[:])
            wv = const_pool.tile([128, NCHUNK, H], BF16)
            nc.sync.dma_start(wv[:], wv_in[:])
            # I_64 living on partitions 64:128 (diag at x = y + 64), used to
            # bounce the k-half of the packed qk projection down to base 0
            ident_hi = const_pool.tile([128, H], BF16)
            nc.gpsimd.memset(ident_hi[:], 0.0)
            nc.gpsimd.affine_select(
                out=ident_hi[:],
                in_=ident_hi[:],
                compare_op=mybir.AluOpType.not_equal,
                fill=1.0,
                base=-H,
                pattern=[[-1, H]],
                channel_multiplier=1,
            )

            # --- 3-stage software pipeline over pairs ---
            # Engine queues are strict FIFO: if pair p's attention matmuls
            # reach the PE queue head while still waiting on DVE casts /
            # exp / mask, they head-of-line-block pair p+1's READY
            # projection matmuls, the PE micro-idles, and the HAM clock
            # gate re-throttles to 1.2 GHz.  Issuing stage A (projections,
            # pair p), stage B (bounce+ST+exp+mask, pair p-1) and stage C
            # (PV+drain, pair p-2) per iteration gives every op a full
            # iteration (~2us) between issue and queue-head arrival.
            S = {}  # live tiles per pair

            def stage_load(pp):
                xt = xt_pool.tile([128, NCHUNK, 2, T], BF16)
                nc.sync.dma_start(xt[:], x_in[pp])
                S[pp] = {"xt": xt}

            def stage_a(pp):
                xt = S[pp]["xt"]
                # qk: weight-stationary, N=512 streams.  v: x-stationary
                # (12 tiny matmuls, FWL weight loads hide under the qk
                # streams) -> v lands as [s, h], no transposes needed.
                qk_ps = proj_ps.tile([128, 2, T], F32, tag="proj")
                v_ps = v_ps_pool.tile([128, 2, 2, H], F32)
                for n in range(NCHUNK):
                    nc.tensor.matmul(
                        qk_ps[:],
                        wqk[:, n, :],
                        xt[:, n],
                        start=(n == 0),
                        stop=(n == NCHUNK - 1),
                    )
                # each (j, m) accumulation group's chunks stay consecutive:
                # a start=True matmul clears the whole PSUM bank's
                # has_written bits, so interleaving groups in one bank
                # corrupts them
                for j in range(2):
                    for m in range(2):
                        for n in range(NCHUNK):
                            nc.tensor.matmul(
                                v_ps[:, j, m, :],
                                xt[:, n, j, bass.ts(m, 128)],
                                wv[:, n, :],
                                start=(n == 0),
                                stop=(n == NCHUNK - 1),
                            )
                qk_sb = proj_sb.tile([128, 2, T], BF16, tag="qk")
                nc.vector.tensor_copy(qk_sb[:], qk_ps[:])
                # v + ones column (col H) for the free softmax row-sum.
                # The slot's first use seeds the whole tile with 1.0; the
                # data columns are overwritten every pair, col H never is.
                v_sb = v_sb_pool.tile([128, 2, 2, H + 1], BF16, tag="v")
                if pp < 4:
                    nc.gpsimd.memset(v_sb[:], 1.0)
                nc.scalar.copy(v_sb[:, :, :, 0:H], v_ps[:])
                S[pp].update(qk_sb=qk_sb, v_sb=v_sb)

            def stage_b1(pp):
                qk_sb = S[pp]["qk_sb"]
                # bounce k (partitions 64:128) down to base 0 via I_64
                k2_ps = kb_ps.tile([H, 2, T], F32, tag="kb")
                nc.tensor.matmul(
                    k2_ps[:],
                    ident_hi[H:128, :],
                    qk_sb[H:128],
                    start=True,
                    stop=True,
                )
                k_sb = proj_sb.tile([H, 2, T], BF16, tag="k")
                nc.vector.tensor_copy(k_sb[:], k2_ps[:])
                S[pp]["k_sb"] = k_sb

            def stage_b2(pp):
                qk_sb = S[pp]["qk_sb"]
                k_sb = S[pp]["k_sb"]
                ps = []
                for j in range(2):
                    qT = qk_sb[0:H, j]        # [64, 256], base partition 0
                    kT = k_sb[:, j]           # [64, 256], base partition 0
                    # ST blocks: [diag0 | diag1 | off-diag]; the two causal
                    # diagonal blocks are adjacent so ONE affine_select masks
                    # both.  diag0+off share the kT[:,0:128] stationary.
                    st = st_ps_pool.tile([128, 3, 128], F32, tag="st")
                    nc.tensor.matmul(
                        st[:, 0, :], kT[:, 0:128], qT[:, 0:128], start=True, stop=True
                    )
                    nc.tensor.matmul(
                        st[:, 2, :], kT[:, 0:128], qT[:, 128:T], start=True, stop=True
                    )
                    nc.tensor.matmul(
                        st[:, 1, :], kT[:, 128:T], qT[:, 128:T], start=True, stop=True
                    )
                    p = p_pool.tile([128, 3, 128], BF16, tag="p")
                    nc.scalar.activation(p[:], st[:], EXP)
                    # zero the causally-invalid lower triangle (s > t) of
                    # both diagonal blocks in one op (keep where t - s >= 0)
                    nc.gpsimd.affine_select(
                        out=p[:, 0:2, :],
                        in_=p[:, 0:2, :],
                        compare_op=mybir.AluOpType.is_ge,
                        fill=0.0,
                        base=0,
                        pattern=[[0, 2], [1, 128]],
                        channel_multiplier=-1,
                    )
                    ps.append(p)
                S[pp]["p"] = ps

            def stage_c(pp):
                v_sb = S[pp]["v_sb"]
                o_ps = o_ps_pool.tile([128, 2, 2, H + 1], F32, tag="o")
                for j in range(2):
                    p = S[pp]["p"][j]
                    nc.tensor.matmul(
                        o_ps[:, j, 0, :], p[:, 0, :], v_sb[:, j, 0, :],
                        start=True, stop=True,
                    )
                    nc.tensor.matmul(
                        o_ps[:, j, 1, :], p[:, 2, :], v_sb[:, j, 0, :],
                        start=True, stop=False,
                    )
                    nc.tensor.matmul(
                        o_ps[:, j, 1, :], p[:, 1, :], v_sb[:, j, 1, :],
                        start=False, stop=True,
                    )
                ob = o_sb_pool.tile([128, 2, 2, H + 1], BF16, tag="ob")
                nc.vector.tensor_copy(ob[:], o_ps[:])
                nc.sync.dma_start(out_d[pp], ob[:])
                del S[pp]

            for i in range(4):
                stage_load(i)
            for it in range(PAIRS + 2):
                if it + 4 < PAIRS:
                    stage_load(it + 4)
                # b1 before A: the chain-critical k cast issues first on the
                # DVE FIFO; stage A's ~1.2us of projection matmuls then sit
                # between the k cast and the ST matmuls that wait on it, so
                # the ST semaphore is satisfied before queue-head arrival
                if 0 <= it - 1 < PAIRS:
                    stage_b1(it - 1)
                if it < PAIRS:
                    stage_a(it)
                if 0 <= it - 1 < PAIRS:
                    stage_b2(it - 1)
                if 0 <= it - 2 < PAIRS:
                    stage_c(it - 2)

    nc.finalize()
    return nc


_CACHED = {}


def _get_nc():
    if "nc" not in _CACHED:
        _CACHED["nc"] = build_bass()
    return _CACHED["nc"]


def prep_inputs(x, Wq, Wk, Wv):
    import ml_dtypes

    bf16 = ml_dtypes.bfloat16
    x = np.ascontiguousarray(x, dtype=np.float32)
    wqk = np.concatenate([Wq * SCALE, Wk], axis=1).astype(np.float32)  # [384, 128]
    wqk_t = np.ascontiguousarray(
        wqk.reshape(NCHUNK, 128, 128).transpose(1, 0, 2).astype(bf16)
    )
    wv_t = np.ascontiguousarray(
        Wv.astype(np.float32).reshape(NCHUNK, 128, H).transpose(1, 0, 2).astype(bf16)
    )

    in_maps = []
    for c in range(N_CORES):
        xs = x[c * BPC : (c + 1) * BPC]  # [64, 256, 384]
        # [pp, j, t, n, p] -> [pp, p, n, j, t]  (partition-major for the DMA)
        xt = np.ascontiguousarray(
            xs.reshape(PAIRS, 2, T, NCHUNK, 128).transpose(0, 4, 3, 1, 2).astype(bf16)
        )
        in_maps.append({"xt": xt, "wqk": wqk_t, "wv": wv_t})
    return in_maps


def postprocess(results):
    outs = []
    for c in range(N_CORES):
        od = results[c]["out"].astype(np.float32)  # [PAIRS, 128p, 2j, 2m, H+1]
        o = od[..., 0:H] / od[..., H : H + 1]      # softmax normalization
        outs.append(o.transpose(0, 2, 3, 1, 4).reshape(BPC, T, H))
    return np.concatenate(outs, axis=0).astype(np.float32)


def kernel(x, Wq, Wk, Wv):
    in_maps = prep_inputs(x, Wq, Wk, Wv)
    res = run_bass_kernel_spmd(_get_nc(), in_maps, core_ids=list(range(N_CORES)))
    return postprocess(res.results)
